# revision 9
# baseline (speedup 1.0000x reference)
"""Trainium2 Bass kernel for nn_Decoder_2688649527663 (LSTM decoder w/ sampling).

Strategy (8 NeuronCores):
- Vocab-parallel: each core owns 4000 of 32000 vocab rows of w_out (resident in
  SBUF as bf16 hi/lo split), computes logits for its shard, softmax stats +
  gumbel-argmax candidates locally; cross-core combine via tiny AllGather.
- LSTM tensor-parallel: each core computes 512 of the 4096 gate rows
  (1/8 of each gate i,f,g,o -> its 128-feature shard of h,c). Full h.T is
  AllGathered (bf16 hi/lo packed) each step for the next-step w_hh matmul and
  the logits matmul.
- All matmuls use bf16 hi/lo 3-product splits accumulated in fp32 PSUM
  (~2^-18 relative error; validated to reproduce the reference sampling
  trajectory exactly in fp32 emulation).
- Gumbel noise is a data-independent constant (threefry from fixed seed 42):
  precomputed on host exactly as jax.random.categorical does, fed per-core.
- Batch M=32 matmuls are packed 4x into PE column strips (tile_position) for
  array utilization; gate partial sums across strips are reduced with a
  stacked-identity selector matmul.
"""
import os
import sys

sys.path.insert(0, "/opt/trn_rl_repo")

import numpy as np
import ml_dtypes

import concourse.bass as bass
import concourse.bacc as bacc
import concourse.tile as tile
from concourse import mybir
from concourse.bass_utils import run_bass_kernel_spmd

NCORES = 8
B, V, EMB, ZD, H = 32, 32000, 512, 512, 1024
VS = V // NCORES          # 4000 vocab per core
HS = H // NCORES          # 128 h-features per core
GS = 4 * HS               # 512 gate rows per core
KH = H // 128             # 8 K-tiles over h
KE = EMB // 128           # 4 K-tiles over emb features
KZ = ZD // 128            # 4 K-tiles over z features

f32 = mybir.dt.float32
bf16 = mybir.dt.bfloat16
i32 = mybir.dt.int32
u32 = mybir.dt.uint32
u8 = mybir.dt.uint8


def build_kernel(T: int):
    nc = bacc.Bacc("TRN2", target_bir_lowering=False, debug=False,
                   num_devices=NCORES, enable_asserts=True)

    # ---------------- I/O ----------------
    d_embhi = nc.dram_tensor("embhi", [V, EMB], bf16, kind="ExternalInput")
    d_emblo = nc.dram_tensor("emblo", [V, EMB], bf16, kind="ExternalInput")
    d_wout = nc.dram_tensor("wout", [128, 2 * KH, VS], bf16, kind="ExternalInput")  # [:, 0:8]=hi, [:, 8:16]=lo
    d_whh = nc.dram_tensor("whh", [128, 2 * KH, GS], bf16, kind="ExternalInput")
    d_wihe = nc.dram_tensor("wihe", [128, 2 * KE, GS], bf16, kind="ExternalInput")
    d_wihz = nc.dram_tensor("wihz", [128, 2 * KZ, GS], bf16, kind="ExternalInput")
    d_zt = nc.dram_tensor("zt", [128, 2 * KZ, B], bf16, kind="ExternalInput")
    d_bsh = nc.dram_tensor("bsh", [1, GS], f32, kind="ExternalInput")
    d_gum = nc.dram_tensor("gum", [T, 128, 1000], f32, kind="ExternalInput")
    d_cb = nc.dram_tensor("cb", [128, 1], f32, kind="ExternalInput")
    d_selsum = nc.dram_tensor("selsum", [128, B], f32, kind="ExternalInput")
    d_selbc = nc.dram_tensor("selbc", [B, 128], f32, kind="ExternalInput")
    d_idf = nc.dram_tensor("idf", [128, 128], f32, kind="ExternalInput")
    d_idb = nc.dram_tensor("idb", [128, 128], bf16, kind="ExternalInput")
    d_sinit = nc.dram_tensor("sinit", [B, 1], i32, kind="ExternalInput")
    d_ones = nc.dram_tensor("ones", [1, B], f32, kind="ExternalInput")

    d_pout = nc.dram_tensor("pout", [T, 128, 1000], f32, kind="ExternalOutput")
    d_sout = nc.dram_tensor("sout", [T, B], i32, kind="ExternalOutput")

    with tile.TileContext(nc) as tc:
        with (
            tc.tile_pool(name="wpool", bufs=1) as wp,       # static weights
            tc.tile_pool(name="sb", bufs=2) as sb,          # per-step tiles
            tc.tile_pool(name="sb1", bufs=1) as sb1,        # single-buffered big tiles
            tc.tile_pool(name="ps", bufs=1, space="PSUM") as ps1,
            tc.tile_pool(name="psL", bufs=2, space="PSUM") as psL,
            tc.tile_pool(name="dr", bufs=2, space="DRAM") as dr,
        ):
            # ---------- static loads ----------
            wout = wp.tile([128, 2 * KH, VS], bf16, tag="wout")
            whh = wp.tile([128, 2 * KH, GS], bf16, tag="whh")
            wihe = wp.tile([128, 2 * KE, GS], bf16, tag="wihe")
            selsum = wp.tile([128, B], f32, tag="selsum")
            selbc = wp.tile([B, 128], f32, tag="selbc")
            idf = wp.tile([128, 128], f32, tag="idf")
            idb = wp.tile([128, 128], bf16, tag="idb")
            cb = wp.tile([128, 1], f32, tag="cb")
            big32 = wp.tile([B, 32], f32, tag="big32")
            ones = wp.tile([1, B], f32, tag="ones")
            nc.sync.dma_start(ones[:], d_ones[:])
            nc.sync.dma_start(wout[:], d_wout[:])
            nc.sync.dma_start(whh[:], d_whh[:])
            nc.sync.dma_start(wihe[:], d_wihe[:])
            nc.sync.dma_start(selsum[:], d_selsum[:])
            nc.sync.dma_start(selbc[:], d_selbc[:])
            nc.sync.dma_start(idf[:], d_idf[:])
            nc.sync.dma_start(idb[:], d_idb[:])
            nc.sync.dma_start(cb[:], d_cb[:])
            nc.gpsimd.memset(big32[:], 1.0e9)

            # ---------- C_z = z @ w_ihz.T + b  (once) ----------
            wihz = wp.tile([128, 2 * KZ, GS], bf16, tag="wihz")
            zt = wp.tile([128, 2 * KZ, B], bf16, tag="zt")
            bsh = wp.tile([1, GS], f32, tag="bsh")
            nc.sync.dma_start(wihz[:], d_wihz[:])
            nc.sync.dma_start(zt[:], d_zt[:])
            nc.sync.dma_start(bsh[:], d_bsh[:])

            zjobs = []
            for k in range(KZ):
                # products hi*hi, hi*lo, lo*hi ; hi at [:, k], lo at [:, KZ+k]
                zjobs.append((zt[:, k, :], wihz[:, k, :]))
                zjobs.append((zt[:, k, :], wihz[:, KZ + k, :]))
                zjobs.append((zt[:, KZ + k, :], wihz[:, k, :]))
            pgz = ps1.tile([128, GS], f32, tag="pg")
            for s in range(4):
                chunk = zjobs[3 * s:3 * s + 3]
                for j, (lt, rt) in enumerate(chunk):
                    nc.tensor.matmul(pgz[32 * s:32 * s + 32, :], lt, rt,
                                     start=(j == 0), stop=(j == len(chunk) - 1),
                                     tile_position=(0, 32 * s))
            pgz_sb = sb1.tile([128, GS], f32, tag="pg_sb")
            nc.scalar.copy(pgz_sb[:], pgz[:])
            psg = ps1.tile([B, GS], f32, tag="psg")
            nc.tensor.matmul(psg[:], selsum[:], pgz_sb[:], start=True, stop=False)
            nc.tensor.matmul(psg[:], ones[:], bsh[:], start=False, stop=True)
            Cz = wp.tile([B, GS], f32, tag="Cz")
            nc.vector.tensor_copy(Cz[:], psg[:])

            # ---------- recurrent state ----------
            s_t = wp.tile([B, 1], i32, tag="s_t")
            nc.sync.dma_start(s_t[:], d_sinit[:])
            c_t = wp.tile([B, HS], f32, tag="c_t")
            nc.gpsimd.memset(c_t[:], 0.0)

            hT = None  # [128, KH, 64] bf16 (hi cols 0:32, lo 32:64) after AG

            for t in range(T):
                # ===== gather emb rows for s_{t-1} and transpose =====
                eg_hi = sb.tile([B, EMB], bf16, tag="eg_hi")
                eg_lo = sb.tile([B, EMB], bf16, tag="eg_lo")
                nc.gpsimd.indirect_dma_start(
                    out=eg_hi[:], out_offset=None, in_=d_embhi[:],
                    in_offset=bass.IndirectOffsetOnAxis(ap=s_t[:, :1], axis=0))
                nc.gpsimd.indirect_dma_start(
                    out=eg_lo[:], out_offset=None, in_=d_emblo[:],
                    in_offset=bass.IndirectOffsetOnAxis(ap=s_t[:, :1], axis=0))
                eT_ps = ps1.tile([128, 2 * KE, 32], bf16, tag="psA")
                for k in range(KE):
                    nc.tensor.transpose(eT_ps[:, k, :],
                                        eg_hi[:, 128 * k:128 * k + 128],
                                        idb[:B, :B])
                    nc.tensor.transpose(eT_ps[:, KE + k, :],
                                        eg_lo[:, 128 * k:128 * k + 128],
                                        idb[:B, :B])
                eT = sb.tile([128, 2 * KE, 32], bf16, tag="eT")
                nc.vector.tensor_copy(eT[:], eT_ps[:])

                # ===== gate matmuls: emb part + h part, 4 column strips =====
                jobs = []
                for k in range(KE):
                    jobs.append((eT[:, k, :], wihe[:, k, :]))
                    jobs.append((eT[:, k, :], wihe[:, KE + k, :]))
                    jobs.append((eT[:, KE + k, :], wihe[:, k, :]))
                if t > 0:
                    for k in range(KH):
                        jobs.append((hT[:, k, 0:32], whh[:, k, :]))
                        jobs.append((hT[:, k, 0:32], whh[:, KH + k, :]))
                        jobs.append((hT[:, k, 32:64], whh[:, k, :]))
                njobs = len(jobs)
                per = (njobs + 3) // 4
                pg = ps1.tile([128, GS], f32, tag="pg")
                for s in range(4):
                    chunk = jobs[per * s:per * s + per]
                    for j, (lt, rt) in enumerate(chunk):
                        nc.tensor.matmul(pg[32 * s:32 * s + 32, :], lt, rt,
                                         start=(j == 0),
                                         stop=(j == len(chunk) - 1),
                                         tile_position=(0, 32 * s))
                pg_sb = sb1.tile([128, GS], f32, tag="pg_sb")
                nc.scalar.copy(pg_sb[:], pg[:])
                psg2 = ps1.tile([B, GS], f32, tag="psg")
                nc.tensor.matmul(psg2[:], selsum[:], pg_sb[:], start=True, stop=True)

                # ===== LSTM cell (local gate order: i, f, o, g) =====
                gt = sb1.tile([B, GS], f32, tag="gt")
                nc.vector.tensor_tensor(out=gt[:], in0=psg2[:], in1=Cz[:],
                                        op=mybir.AluOpType.add)
                sg = sb.tile([B, 384], f32, tag="sg")
                nc.scalar.activation(sg[:], gt[:, 0:384],
                                     mybir.ActivationFunctionType.Sigmoid)
                tg = sb.tile([B, HS], f32, tag="tg")
                nc.scalar.activation(tg[:], gt[:, 384:512],
                                     mybir.ActivationFunctionType.Tanh)
                t1 = sb.tile([B, HS], f32, tag="t1")
                nc.vector.tensor_tensor(out=t1[:], in0=sg[:, 128:256], in1=c_t[:],
                                        op=mybir.AluOpType.mult)
                t2 = sb.tile([B, HS], f32, tag="t2")
                nc.vector.tensor_tensor(out=t2[:], in0=sg[:, 0:128], in1=tg[:],
                                        op=mybir.AluOpType.mult)
                c_new = sb.tile([B, HS], f32, tag="c_new")
                nc.vector.tensor_tensor(out=c_new[:], in0=t1[:], in1=t2[:],
                                        op=mybir.AluOpType.add)
                tc_t = sb.tile([B, HS], f32, tag="tc_t")
                nc.scalar.activation(tc_t[:], c_new[:],
                                     mybir.ActivationFunctionType.Tanh)
                h_new = sb.tile([B, HS], f32, tag="h_new")
                nc.vector.tensor_tensor(out=h_new[:], in0=sg[:, 256:384],
                                        in1=tc_t[:], op=mybir.AluOpType.mult)
                # keep c for next step
                nc.vector.tensor_copy(c_t[:], c_new[:])

                # ===== h -> h.T, split hi/lo, AllGather =====
                hT_ps = ps1.tile([128, B], f32, tag="psA")
                nc.tensor.transpose(hT_ps[:], h_new[:], idf[:B, :B])
                hpack = sb.tile([128, 64], bf16, tag="hpack")
                nc.vector.tensor_copy(hpack[:, 0:32], hT_ps[:])
                nc.vector.tensor_tensor(out=hpack[:, 32:64], in0=hT_ps[:],
                                        in1=hpack[:, 0:32],
                                        op=mybir.AluOpType.subtract)
                hb_in = dr.tile([128, 64], bf16, tag="hb_in")
                hb_out = dr.tile([128 * NCORES, 64], bf16, tag="hb_out")
                nc.gpsimd.dma_start(hb_in[:], hpack[:])
                nc.gpsimd.collective_compute(
                    "AllGather", mybir.AluOpType.bypass,
                    replica_groups=[list(range(NCORES))],
                    ins=[hb_in.opt()], outs=[hb_out.opt()])
                hT = sb.tile([128, KH, 64], bf16, tag="hT")
                nc.sync.dma_start(
                    hT[:], hb_out[:].rearrange("(k p) c -> p k c", p=128))

                # ===== logits: 8 chunks x (3 products x 8 K) =====
                Lp0 = psL.tile([128, 500], f32, tag="Lp0")
                Lp1 = psL.tile([128, 500], f32, tag="Lp1")
                Lp = [Lp0, Lp1]
                for s in range(4):
                    for bank in range(2):
                        co = (s * 1000 + bank * 500)
                        n = 0
                        for k in range(KH):
                            for (lt, rt) in (
                                (hT[:, k, 0:32], wout[:, k, co:co + 500]),
                                (hT[:, k, 0:32], wout[:, KH + k, co:co + 500]),
                                (hT[:, k, 32:64], wout[:, k, co:co + 500]),
                            ):
                                nc.tensor.matmul(
                                    Lp[bank][32 * s:32 * s + 32, :], lt, rt,
                                    start=(n == 0), stop=(n == 3 * KH - 1),
                                    tile_position=(0, 32 * s))
                                n += 1

                # ===== softmax stats + gumbel argmax =====
                g_t = sb.tile([128, 1000], f32, tag="g_t")
                nc.sync.dma_start(g_t[:], d_gum[t])
                A = sb.tile([128, 1000], f32, tag="A")
                ex = sb1.tile([128, 1000], f32, tag="ex")
                se = sb.tile([128, 2], f32, tag="se")
                for bank in range(2):
                    sl = slice(500 * bank, 500 * bank + 500)
                    nc.vector.tensor_tensor(out=A[:, sl], in0=Lp[bank][:],
                                            in1=g_t[:, sl],
                                            op=mybir.AluOpType.add)
                    nc.scalar.activation(ex[:, sl], Lp[bank][:],
                                         mybir.ActivationFunctionType.Exp,
                                         accum_out=se[:, bank:bank + 1])
                mx8 = sb.tile([128, 8], f32, tag="mx8")
                ix8 = sb.tile([128, 8], u32, tag="ix8")
                nc.vector.max(mx8[:], A[:])
                nc.vector.max_index(ix8[:], mx8[:], A[:])

                st = sb.tile([128, 3], f32, tag="st")
                nc.vector.tensor_copy(st[:, 0:1], mx8[:, 0:1])
                ixf = sb.tile([128, 1], f32, tag="ixf")
                nc.vector.tensor_copy(ixf[:], ix8[:, 0:1])
                nc.vector.tensor_tensor(out=st[:, 1:2], in0=ixf[:], in1=cb[:],
                                        op=mybir.AluOpType.add)
                nc.vector.tensor_tensor(out=st[:, 2:3], in0=se[:, 0:1],
                                        in1=se[:, 1:2], op=mybir.AluOpType.add)
                stT = ps1.tile([3, 128], f32, tag="psB")
                nc.tensor.transpose(stT[:], st[:], idf[:])
                stT_sb = sb.tile([3, 128], f32, tag="stT_sb")
                nc.vector.tensor_copy(stT_sb[:], stT[:])

                # ===== stats AllGather (raw [3,128]) + combine =====
                pk_in = dr.tile([3, 128], f32, tag="pk_in")
                pk_out = dr.tile([3 * NCORES, 128], f32, tag="pk_out")
                nc.gpsimd.dma_start(pk_in[:], stT_sb[:])
                nc.gpsimd.collective_compute(
                    "AllGather", mybir.AluOpType.bypass,
                    replica_groups=[list(range(NCORES))],
                    ins=[pk_in.opt()], outs=[pk_out.opt()])
                # reload with batch on partitions: [32(b), 8(r), 3(m), 4(s)]
                rk = sb.tile([B, NCORES, 3, 4], f32, tag="rk")
                nc.sync.dma_start(
                    rk[:], pk_out[:].rearrange("(r m) (s b) -> b r m s", m=3, b=B))

                gmax = sb.tile([B, 1], f32, tag="gmax")
                nc.vector.tensor_reduce(out=gmax[:], in_=rk[:, :, 0, :],
                                        axis=mybir.AxisListType.XY,
                                        op=mybir.AluOpType.max)
                msk8 = sb.tile([B, NCORES, 4], u8, tag="msk8")
                nc.vector.tensor_scalar(out=msk8[:], in0=rk[:, :, 0, :],
                                        scalar1=gmax[:], scalar2=None,
                                        op0=mybir.AluOpType.is_ge)
                sel8 = sb.tile([B, NCORES, 4], f32, tag="sel8")
                nc.vector.select(sel8[:], msk8[:], rk[:, :, 1, :],
                                 big32[:].rearrange("b (r s) -> b r s", s=4))
                s_f = sb.tile([B, 1], f32, tag="s_f")
                nc.vector.tensor_reduce(out=s_f[:], in_=sel8[:],
                                        axis=mybir.AxisListType.XY,
                                        op=mybir.AluOpType.min)
                nc.vector.tensor_copy(s_t[:], s_f[:])
                nc.sync.dma_start(d_sout[t], s_t[:])

                # ===== p = ex * (1 / sum_exp) =====
                D = sb.tile([B, 1], f32, tag="D")
                nc.vector.tensor_reduce(out=D[:], in_=rk[:, :, 2, :],
                                        axis=mybir.AxisListType.XY,
                                        op=mybir.AluOpType.add)
                rD = sb.tile([B, 1], f32, tag="rD")
                nc.vector.reciprocal(rD[:], D[:])
                rD_ps = ps1.tile([128, 1], f32, tag="psB")
                nc.tensor.matmul(rD_ps[:], selbc[:], rD[:], start=True, stop=True)
                rDb = sb.tile([128, 1], f32, tag="rDb")
                nc.vector.tensor_copy(rDb[:], rD_ps[:])
                p_t = sb.tile([128, 1000], f32, tag="A")
                nc.vector.tensor_scalar(out=p_t[:], in0=ex[:], scalar1=rDb[:],
                                        scalar2=None, op0=mybir.AluOpType.mult)
                nc.sync.dma_start(d_pout[t], p_t[:])

    nc.compile()
    return nc


def split_bf16(x):
    hi = np.asarray(x, np.float32).astype(ml_dtypes.bfloat16)
    lo = (np.asarray(x, np.float32) - hi.astype(np.float32)).astype(ml_dtypes.bfloat16)
    return hi, lo


def _ktile(mat):
    """[K, N] fp32 -> ([128, k, N] hi, [128, k, N] lo) bf16"""
    K, N = mat.shape
    k = K // 128
    hi, lo = split_bf16(mat)
    hi = hi.reshape(k, 128, N).transpose(1, 0, 2)
    lo = lo.reshape(k, 128, N).transpose(1, 0, 2)
    return np.concatenate([hi, lo], axis=1)  # [128, 2k, N]


def _gumbel_host(T):
    import jax
    import jax.numpy as jnp
    cpu = jax.local_devices(backend="cpu")[0]
    with jax.default_device(cpu):
        base = jax.random.key(42)
        g = [np.asarray(jax.random.gumbel(jax.random.fold_in(base, t), (B, V),
                                          jnp.float32)) for t in range(T)]
    return np.stack(g)  # [T, B, V]


def prepare_in_maps(z, emb, w_ih, w_hh, b, w_out, b_out, start_id, T):
    z = np.asarray(z, np.float32)
    emb = np.asarray(emb, np.float32)
    w_ih = np.asarray(w_ih, np.float32)
    w_hh = np.asarray(w_hh, np.float32)
    b = np.asarray(b, np.float32)
    w_out = np.asarray(w_out, np.float32)
    b_out = np.asarray(b_out, np.float32)
    assert np.abs(b_out).max() == 0.0, "kernel compiled for b_out == 0"

    G = _gumbel_host(T)  # [T, B, V]

    emb_hi, emb_lo = split_bf16(emb)
    idf = np.eye(128, dtype=np.float32)
    idb = np.eye(128, dtype=np.float32).astype(ml_dtypes.bfloat16)
    selsum = np.tile(np.eye(B, dtype=np.float32), (4, 1))      # [128, 32]
    selbc = selsum.T.copy()                                    # [32, 128]
    sinit = np.full((B, 1), int(start_id), np.int32)

    in_maps = []
    for r in range(NCORES):
        rows = np.concatenate([
            np.arange(g * H + r * HS, g * H + (r + 1) * HS) for g in (0, 1, 3, 2)
        ])  # local gate order i,f,o,g
        wihe_r = _ktile(w_ih[rows, :EMB].T)        # [128, 8, 512]
        wihz_r = _ktile(w_ih[rows, EMB:].T)
        whh_r = _ktile(w_hh[rows, :].T)            # [128, 16, 512]
        wout_r = _ktile(w_out[r * VS:(r + 1) * VS, :].T)   # [128, 16, 4000]
        zt_r = _ktile(z.T)                          # [128, 8, 32]
        bsh_r = b[rows][None, :].astype(np.float32)
        cb_r = (r * VS + 1000 * (np.arange(128) // 32))[:, None].astype(np.float32)
        # gumbel: [T, B, VS] -> [T, 128(s,b), 1000(bank,pos)]
        g_r = G[:, :, r * VS:(r + 1) * VS].reshape(T, B, 4, 2, 500)
        g_r = np.ascontiguousarray(g_r.transpose(0, 2, 1, 3, 4)).reshape(T, 128, 1000)
        in_maps.append(dict(
            embhi=emb_hi, emblo=emb_lo, wout=wout_r, whh=whh_r, wihe=wihe_r,
            wihz=wihz_r, zt=zt_r, bsh=bsh_r, gum=g_r, cb=cb_r,
            selsum=selsum, selbc=selbc, idf=idf, idb=idb, sinit=sinit,
            ones=np.ones((1, B), np.float32),
        ))
    return in_maps


def unpack_outputs(results, T):
    s_out = results[0]["sout"].astype(np.int32)          # [T, B]
    p_parts = []
    for r in range(NCORES):
        p_r = results[r]["pout"]                         # [T, 128, 1000]
        p_r = p_r.reshape(T, 4, B, 2, 500).transpose(2, 0, 1, 3, 4)
        p_parts.append(p_r.reshape(B, T, VS))
    p_full = np.concatenate(p_parts, axis=2)             # [B, T, V]
    return s_out.T.copy(), p_full


def kernel(z, emb, w_ih, w_hh, b, w_out, b_out, start_id, maxtime):
    T = int(maxtime)
    in_maps = prepare_in_maps(z, emb, w_ih, w_hh, b, w_out, b_out, start_id, T)
    nc = build_kernel(T)
    res = run_bass_kernel_spmd(nc, in_maps, core_ids=list(range(NCORES)))
    global _LAST_RES
    _LAST_RES = res
    return unpack_outputs(res.results, T)
